# revision 1
# baseline (speedup 1.0000x reference)
"""Multi-head self-attention with positional bias, sharded over 8 NeuronCores.

Sharding: head-parallel. Core h computes head h for all batches:
  q/k/v projections with the head's weight slices, scores + softmax with the
  head's pos_bias slice, and the partial output  o_h @ Wout[h*64:(h+1)*64, :].
The full output is the sum of the 8 partials (row-parallel Wout).

Device kernel math (per core):
  - query is supplied pre-transposed (qT [D, B*N]) so the contraction dim of
    every projection lands on SBUF partitions.
  - scores are computed TRANSPOSED: ST[j, i] = bias[i, j] + k_j . q_i, so exp's
    output P~[j, i] is directly the layout the attention*V matmul needs (no P
    transposes). The bias lands in PSUM via an identity matmul (start=True),
    the qk matmul accumulates on top.
  - exp skips max-subtraction: scores are ~N(0, 2) (bounded), exp is safe in
    fp32 and softmax is shift-invariant.
  - softmax denominator: ones column appended to v (safe mode) or packed
    ones-matmuls (packed mode); normalization is deferred to the PSUM
    evacuation after the Wout matmul (per-partition scalar multiply).
  - all matmuls run in float32r (fp22 mantissa truncation, full PE speed at
    free-dim >= 256, fp32 accumulation): rel err ~1e-4.
"""

import numpy as np
from contextlib import ExitStack

import concourse.bass as bass
import concourse.bacc as bacc
import concourse.mybir as mybir
import concourse.tile as tile
from concourse.bass_utils import run_bass_kernel_spmd
from concourse.masks import make_identity

HEADS = 8
DH = 64
B, N, D = 4, 2048, 512
SCALE = DH ** -0.5
N_CORES = 8
PACKED = False  # shared-PSUM-bank packing tricks (col-strip oT, 4-way denom)

F32 = mybir.dt.float32
F32R = mybir.dt.float32r


def build_nc(b=B, n=N, d=D, packed=PACKED, n_cores=1):
    """Build the per-core Bass program. All cores run the same program (SPMD);
    per-head differences come in through the input tensors."""
    assert b % 2 == 0 and n % 512 == 0 and d % 128 == 0
    T = b * n           # total tokens
    CC = d // 128       # contraction chunks for the projections
    NJ = n // 128       # key tiles (j)
    NIC = n // 512      # query chunks of 512 (i)
    assert NIC % 2 == 0
    NIP = NIC // 2      # i-groups of 1024 (one exp op each)
    NPAIR = b // 2
    IC = 512
    VW = 64 if packed else 65  # v block width (safe mode: +1 ones column)

    nc = bacc.Bacc("TRN2", target_bir_lowering=False, debug=False,
                   num_devices=n_cores)
    qT = nc.declare_dram_parameter("qT", [d, T], F32R, isOutput=False)
    biasT = nc.declare_dram_parameter("biasT", [n, n], F32R, isOutput=False)
    wq = nc.declare_dram_parameter("wq", [d, DH], F32R, isOutput=False)
    wk = nc.declare_dram_parameter("wk", [d, DH], F32R, isOutput=False)
    wv = nc.declare_dram_parameter("wv", [d, DH], F32R, isOutput=False)
    wout = nc.declare_dram_parameter("wout", [DH, d], F32R, isOutput=False)
    out = nc.declare_dram_parameter("out", [T, d], F32, isOutput=True)

    with ExitStack() as ctx:
        tc = ctx.enter_context(tile.TileContext(nc))

        const = ctx.enter_context(tc.tile_pool(name="const", bufs=1))
        qk_pool = ctx.enter_context(tc.tile_pool(name="qkT", bufs=1))
        v_pool = ctx.enter_context(tc.tile_pool(name="v", bufs=1))
        ot_sb_pool = ctx.enter_context(tc.tile_pool(name="ot_sb", bufs=1))
        p_pool = ctx.enter_context(tc.tile_pool(name="pexp", bufs=4))
        out_pool = ctx.enter_context(tc.tile_pool(name="osb", bufs=6))

        ident_f32 = const.tile([128, 128], F32, tag="ident_f32")
        make_identity(nc, ident_f32)
        ident = const.tile([128, 128], F32R, tag="ident")
        nc.vector.tensor_copy(ident, ident_f32)
        zbias = const.tile([128, 1], F32, tag="zbias")
        nc.vector.memset(zbias, 0.0)
        ones16 = const.tile([128, 16], F32, tag="ones16")
        nc.vector.memset(ones16, 1.0)
        if packed:
            ones32 = const.tile([128, 32], F32R, tag="ones32")
            nc.vector.tensor_copy(ones32[:, 0:16], ones16)
            nc.vector.tensor_copy(ones32[:, 16:32], ones16)

        w_sb = {}
        for name, w in (("wq", wq), ("wk", wk), ("wv", wv)):
            t = const.tile([128, CC, DH], F32R, tag=name)
            nc.sync.dma_start(out=t, in_=w[:, :].rearrange("(c p) e -> p c e", p=128))
            w_sb[name] = t
        wout_sb = const.tile([128, d], F32R, tag="wout")
        nc.sync.dma_start(out=wout_sb[0:64, :], in_=wout[:, :])
        nc.sync.dma_start(out=wout_sb[64:128, :], in_=wout[:, :])

        qT_sb = [qk_pool.tile([128, n], F32R, tag=f"qT{p}", name=f"qT{p}") for p in range(NPAIR)]
        kT_sb = [qk_pool.tile([128, n], F32R, tag=f"kT{p}", name=f"kT{p}") for p in range(NPAIR)]
        v_sb = [v_pool.tile([128, NJ * VW], F32R, tag=f"v{bb}", name=f"v{bb}") for bb in range(b)]
        if not packed:
            for bb in range(b):
                ones_cols = v_sb[bb].rearrange("p (t w) -> p t w", w=VW)[:, :, DH:VW]
                nc.vector.tensor_copy(ones_cols, ones16[:, 0:NJ].rearrange("p (t o) -> p t o", o=1))
        ot_sb = [ot_sb_pool.tile([128, n], F32R, tag=f"ot{p}", name=f"ot{p}") for p in range(NPAIR)]

        # denominator staging: row bb lives at partition 32*bb (engines need
        # 32-aligned partition bases)
        den_all = const.tile([32 * (b - 1) + 1, n], F32, tag="den_all")
        den_sb = [den_all[32 * bb:32 * bb + 1, :] for bb in range(b)]
        recip_in = [const.tile([128, NJ], F32, tag=f"recip_in{bb}", name=f"ri{bb}")
                    for bb in range(b)]
        recip_sb = [const.tile([128, NJ], F32, tag=f"recip_sb{bb}", name=f"rs{bb}")
                    for bb in range(b)]

        # ---------------- projections (per batch) ----------------
        HN = max(n // 4, 512)  # qt chunk width (>= one projection rhs slice)
        NQ = n // HN
        with tc.tile_pool(name="qt", bufs=3 * CC) as qt_pool, \
             tc.tile_pool(name="pqk", bufs=4, space="PSUM") as pqk_pool, \
             tc.tile_pool(name="pv", bufs=4, space="PSUM") as pv_pool:
            for bb in range(b):
                pair, lb = bb // 2, bb % 2
                rows = slice(64 * lb, 64 * lb + 64)
                for hh in range(NQ):
                    qt_c = []
                    for c in range(CC):
                        t = qt_pool.tile([128, HN], F32R, tag="qt", name="qtc")
                        nc.sync.dma_start(
                            out=t, in_=qT[c * 128:(c + 1) * 128,
                                          bb * n + hh * HN: bb * n + (hh + 1) * HN])
                        qt_c.append(t)
                    for wname, dest in (("wq", qT_sb[pair]), ("wk", kT_sb[pair])):
                        for hic in range(HN // IC):
                            icc = (hh * HN + hic * IC) // IC
                            ps = pqk_pool.tile([64, IC], F32, tag="pqk")
                            for c in range(CC):
                                nc.tensor.matmul(
                                    ps, lhsT=w_sb[wname][:, c, :],
                                    rhs=qt_c[c][:, hic * IC:(hic + 1) * IC],
                                    start=(c == 0), stop=(c == CC - 1))
                            nc.vector.tensor_copy(dest[rows, icc * IC:(icc + 1) * IC], ps)
                    for htt in range(HN // 128):
                        tt = (hh * HN + htt * 128) // 128
                        psv = pv_pool.tile([128, DH], F32, tag="pv")
                        for c in range(CC):
                            nc.tensor.matmul(
                                psv, lhsT=qt_c[c][:, htt * 128:(htt + 1) * 128],
                                rhs=w_sb["wv"][:, c, :],
                                start=(c == 0), stop=(c == CC - 1))
                        nc.vector.tensor_copy(v_sb[bb][:, tt * VW: tt * VW + DH], psv)

        # ---------------- scores + softmax + P~^T V ----------------
        with tc.tile_pool(name="bias", bufs=NJ) as bias_pool, \
             tc.tile_pool(name="st", bufs=2, space="PSUM") as st_pool, \
             tc.tile_pool(name="ot", bufs=2 if packed else 4, space="PSUM") as ot_pool, \
             tc.tile_pool(name="dn", bufs=1, space="PSUM") as dn_pool:
            for ip in range(NIP):
                bias_t = []
                for jt in range(NJ):
                    t = bias_pool.tile([128, 2 * IC], F32R, tag="bias")
                    nc.sync.dma_start(
                        out=t, in_=biasT[jt * 128:(jt + 1) * 128, ip * 2 * IC:(ip + 1) * 2 * IC])
                    bias_t.append(t)
                for pair in range(NPAIR):
                    if packed:
                        ot_ps = [ot_pool.tile([128, IC], F32, tag="ot", name="otp")
                                 for _ in range(2)]
                        dn_ps = dn_pool.tile([128, IC], F32, tag="dn")
                    else:
                        ot_ps = {(lb, il): ot_pool.tile([65, IC], F32, tag="ot", name="otp")
                                 for lb in range(2) for il in range(2)}
                    for jt in range(NJ):
                        for lb in range(2):
                            bb = 2 * pair + lb
                            rows = slice(64 * lb, 64 * lb + 64)
                            st = st_pool.tile([128, 2 * IC], F32, tag="st")
                            for il in range(2):
                                cols = slice(il * IC, (il + 1) * IC)
                                ic = ip * 2 + il
                                nc.tensor.matmul(
                                    st[:, cols], lhsT=ident, rhs=bias_t[jt][:, cols],
                                    start=True, stop=False)
                                nc.tensor.matmul(
                                    st[:, cols],
                                    lhsT=kT_sb[pair][rows, jt * 128:(jt + 1) * 128],
                                    rhs=qT_sb[pair][rows, ic * IC:(ic + 1) * IC],
                                    start=False, stop=True)
                            pexp = p_pool.tile([128, 2 * IC], F32R, tag="pexp")
                            nc.scalar.activation(
                                pexp, st, mybir.ActivationFunctionType.Exp, bias=zbias)
                            for il in range(2):
                                pcols = slice(il * IC, (il + 1) * IC)
                                if packed:
                                    nc.tensor.matmul(
                                        ot_ps[il][rows, :],
                                        lhsT=v_sb[bb][:, jt * VW: jt * VW + DH],
                                        rhs=pexp[:, pcols],
                                        start=(jt == 0 and lb == 0),
                                        stop=(jt == NJ - 1 and lb == 1),
                                        skip_group_check=True)
                                    s_idx = il * 2 + lb
                                    nc.tensor.matmul(
                                        dn_ps[32 * s_idx: 32 * s_idx + 32, :],
                                        lhsT=ones32, rhs=pexp[:, pcols],
                                        start=(jt == 0 and s_idx == 0),
                                        stop=(jt == NJ - 1 and s_idx == 3),
                                        tile_position=(0, 32 * s_idx),
                                        skip_group_check=True)
                                else:
                                    nc.tensor.matmul(
                                        ot_ps[(lb, il)],
                                        lhsT=v_sb[bb][:, jt * VW: jt * VW + VW],
                                        rhs=pexp[:, pcols],
                                        start=(jt == 0), stop=(jt == NJ - 1))
                    # evacuate oT + denominators for this (ip, pair)
                    for il in range(2):
                        ic = ip * 2 + il
                        ccols = slice(ic * IC, (ic + 1) * IC)
                        if packed:
                            for lb in range(2):
                                s_idx = il * 2 + lb
                                bb = 2 * pair + lb
                                nc.vector.tensor_copy(
                                    den_sb[bb][0:1, ccols],
                                    dn_ps[32 * s_idx: 32 * s_idx + 1, :])
                            nc.vector.tensor_copy(ot_sb[pair][:, ccols], ot_ps[il])
                        else:
                            for lb in range(2):
                                bb = 2 * pair + lb
                                rows = slice(64 * lb, 64 * lb + 64)
                                nc.vector.tensor_copy(
                                    den_sb[bb][0:1, ccols], ot_ps[(lb, il)][64:65, :])
                                nc.vector.tensor_copy(
                                    ot_sb[pair][rows, ccols], ot_ps[(lb, il)][0:64, :])

        # denominator rows -> per-token-tile columns (via DRAM bounce), reciprocal
        for bb in range(b):
            den_dram = nc.dram_tensor(f"den_dram{bb}", [n], F32)
            nc.sync.dma_start(out=den_dram[:], in_=den_sb[bb][0:1, :])
            nc.sync.dma_start(
                out=recip_in[bb],
                in_=den_dram[:].rearrange("(t p) -> p t", p=128))
            nc.vector.reciprocal(recip_sb[bb], recip_in[bb])

        # ---------------- output projection ----------------
        with tc.tile_pool(name="po", bufs=6, space="PSUM") as po_pool:
            for pair in range(NPAIR):
                for tg in range(NJ):
                    for lb in range(2):
                        bb = 2 * pair + lb
                        rows = slice(64 * lb, 64 * lb + 64)
                        po = po_pool.tile([128, d], F32, tag="po")
                        nc.tensor.matmul(
                            po, lhsT=ot_sb[pair][rows, tg * 128:(tg + 1) * 128],
                            rhs=wout_sb[rows, :], start=True, stop=True)
                        osb = out_pool.tile([128, d], F32, tag="osb")
                        nc.vector.tensor_scalar_mul(
                            osb, po, recip_sb[bb][:, tg: tg + 1])
                        nc.sync.dma_start(
                            out=out[bb * n + tg * 128: bb * n + (tg + 1) * 128, :],
                            in_=osb)
    nc.compile()
    return nc


def make_in_maps(query, pos_bias, Wq, Wk, Wv, Wout, n_cores=N_CORES):
    """Host-side sharding/layout prep. Head h -> core h."""
    query = np.asarray(query, dtype=np.float32)
    pos_bias = np.asarray(pos_bias, dtype=np.float32)
    Wq = np.asarray(Wq, dtype=np.float32)
    Wk = np.asarray(Wk, dtype=np.float32)
    Wv = np.asarray(Wv, dtype=np.float32)
    Wout = np.asarray(Wout, dtype=np.float32)

    b, n, d = query.shape
    qT = np.ascontiguousarray(query.reshape(b * n, d).T)
    wq_s = Wq * np.float32(SCALE)
    in_maps = []
    for h in range(n_cores):
        sl = slice(h * DH, (h + 1) * DH)
        in_maps.append({
            "qT": qT,
            "biasT": np.ascontiguousarray(pos_bias[h].T),
            "wq": np.ascontiguousarray(wq_s[:, sl]),
            "wk": np.ascontiguousarray(Wk[:, sl]),
            "wv": np.ascontiguousarray(Wv[:, sl]),
            "wout": np.ascontiguousarray(Wout[sl, :]),
        })
    return in_maps


def run_device(in_maps, b=B, n=N, d=D, packed=PACKED, trace=False, **kw):
    nc = build_nc(b, n, d, packed, n_cores=len(in_maps))
    return run_bass_kernel_spmd(nc, in_maps, list(range(len(in_maps))), trace=trace, **kw)


def assemble(results, b=B, n=N, d=D):
    acc = np.zeros((b * n, d), dtype=np.float32)
    for r in results:
        acc += r["out"]
    return acc.reshape(b, n, d)


def kernel(query, pos_bias, Wq, Wk, Wv, Wout):
    in_maps = make_in_maps(query, pos_bias, Wq, Wk, Wv, Wout)
    res = run_device(in_maps)
    return assemble(res.results)



# revision 17
# speedup vs baseline: 1.5744x; 1.5744x over previous
"""Multi-head self-attention with positional bias, sharded over 8 NeuronCores.

Sharding: head-parallel. Core h computes head h for all batches:
  q/k/v projections with the head's weight slices, scores + softmax with the
  head's pos_bias slice, and the partial output  o_h @ Wout[h*64:(h+1)*64, :].
The full output is the sum of the 8 partials (row-parallel Wout).

Device kernel math (per core):
  - query is supplied pre-transposed (qT [D, B*N], bf16) so the contraction
    dim of every projection lands on SBUF partitions.
  - scores are computed TRANSPOSED: ST[j, i] = k_j . q_i (scaled), so exp's
    output P~[j, i] is directly the layout the attention*V matmul needs.
  - the positional bias is folded in multiplicatively AFTER exp:
      exp(qk + bias) = exp(qk) * exp(bias)
    with exp(bias) precomputed on the host (bf16, SBUF-resident for the whole
    kernel). This removes the PE-side bias injection entirely; the multiply
    runs on the DVE in bf16 (2x perf mode).
  - exp skips max-subtraction: qk scores are ~N(0, 1), exp is safe in fp32/bf16
    and softmax is shift-invariant.
  - softmax denominator: a ones column appended to v, so attention*V's 65th
    output row is sum_j P~b[j, i]; normalization is deferred to the PSUM
    evacuation after the Wout matmul (per-partition scalar multiply).
  - projections and attention*V run in bf16 (1 cyc/row on PE at any free dim,
    fp32 PSUM accumulation); scores qk runs in float32r (full q/k precision).
  - PSUM is statically partitioned (proj 2 banks -> reused by output proj,
    scores 4, attn*V accum 2) so the score pipeline overlaps the projections
    and the per-(pair, ig) output projections overlap later score groups.
"""

import numpy as np
from contextlib import ExitStack

import ml_dtypes

import concourse.bass as bass
import concourse.bacc as bacc
import concourse.mybir as mybir
import concourse.tile as tile
from concourse.bass_utils import run_bass_kernel_spmd
from concourse.masks import make_identity

HEADS = 8
DH = 64
B, N, D = 4, 2048, 512
SCALE = DH ** -0.5
N_CORES = 8
PACKED = False  # unused; kept for test.py compatibility

F32 = mybir.dt.float32
F32R = mybir.dt.float32r
BF16 = mybir.dt.bfloat16
BF = ml_dtypes.bfloat16


def build_nc(b=B, n=N, d=D, packed=False, n_cores=1):
    """Build the per-core Bass program. All cores run the same program (SPMD);
    per-head differences come in through the input tensors."""
    assert b % 2 == 0 and n % 512 == 0 and d % 128 == 0
    T = b * n           # total tokens
    CC = d // 128       # contraction chunks for the projections
    NJ = n // 128       # key tiles (j)
    IC = 512            # matmul output column chunk (one PSUM bank)
    IG = min(n, 1024)   # exp/mult group width (2 PSUM banks)
    NIG = n // IG
    NIL = IG // IC      # IC chunks per group
    NPAIR = b // 2
    VW = DH + 1         # v block width (+1 ones column for the denominator)
    TPG = IG // 128     # token tiles per i-group

    nc = bacc.Bacc("TRN2", target_bir_lowering=False, debug=False,
                   num_devices=n_cores)
    qT = nc.declare_dram_parameter("qT", [d, T], BF16, isOutput=False)
    ebiasT = nc.declare_dram_parameter("ebiasT", [n, n], BF16, isOutput=False)
    wq = nc.declare_dram_parameter("wq", [d, DH], BF16, isOutput=False)
    wk = nc.declare_dram_parameter("wk", [d, DH], BF16, isOutput=False)
    wv = nc.declare_dram_parameter("wv", [d, DH], BF16, isOutput=False)
    wout = nc.declare_dram_parameter("wout", [DH, d], F32R, isOutput=False)
    out = nc.declare_dram_parameter("out", [T, d], BF16, isOutput=True)

    with ExitStack() as ctx:
        tc = ctx.enter_context(tile.TileContext(nc))

        const = ctx.enter_context(tc.tile_pool(name="const", bufs=1))
        eb_pool = ctx.enter_context(tc.tile_pool(name="ebias", bufs=1))
        qk_pool = ctx.enter_context(tc.tile_pool(name="qkT", bufs=1))
        v_pool = ctx.enter_context(tc.tile_pool(name="v", bufs=1))
        ot_sb_pool = ctx.enter_context(tc.tile_pool(name="ot_sb", bufs=1))
        p_pool = ctx.enter_context(tc.tile_pool(name="pexp", bufs=4))
        pb_pool = ctx.enter_context(tc.tile_pool(name="pexpb", bufs=4))
        out_pool = ctx.enter_context(tc.tile_pool(name="osb", bufs=2))
        # PSUM: st (4 banks) + ot (2) stay open for the whole kernel; the
        # projection pool (2 banks) closes before the output-projection pool
        # (2 banks) opens, so the static footprint never exceeds 8 banks.
        st_pool = ctx.enter_context(tc.tile_pool(name="st", bufs=2, space="PSUM"))
        ot_pool = ctx.enter_context(tc.tile_pool(name="ot", bufs=2, space="PSUM"))

        ones16 = const.tile([128, 16], BF16, tag="ones16")
        nc.vector.memset(ones16, 1.0)
        ident_f32 = const.tile([128, 128], F32, tag="ident_f32")
        make_identity(nc, ident_f32)
        ident = const.tile([128, 128], F32R, tag="ident")
        nc.vector.tensor_copy(ident, ident_f32)

        w_sb = {}
        for name, w in (("wq", wq), ("wk", wk), ("wv", wv)):
            t = const.tile([128, CC, DH], BF16, tag=name)
            nc.sync.dma_start(out=t, in_=w[:, :].rearrange("(c p) e -> p c e", p=128))
            w_sb[name] = t
        wout_sb = const.tile([64, d], F32R, tag="wout")
        nc.sync.dma_start(out=wout_sb[0:64, :], in_=wout[:, :])

        # full exp(bias) head-slice, resident in SBUF: [128, NJ, n] bf16.
        # Loads are staggered through the early emission (see eb_feed) so they
        # don't monopolize the DMA engines ahead of the qt loads.
        ebias_sb = eb_pool.tile([128, NJ, n], BF16, tag="ebias")

        def _eb_load(jt):
            nc.sync.dma_start(out=ebias_sb[:, jt, :],
                              in_=ebiasT[jt * 128:(jt + 1) * 128, :])
        eb_pending = list(range(NJ))

        def eb_feed(k=1):
            for _ in range(k):
                if eb_pending:
                    _eb_load(eb_pending.pop(0))

        qT_sb = [qk_pool.tile([128, n], F32R, tag=f"qT{p}", name=f"qT{p}") for p in range(NPAIR)]
        kT_sb = [qk_pool.tile([128, n], F32R, tag=f"kT{p}", name=f"kT{p}") for p in range(NPAIR)]
        v_sb = [v_pool.tile([128, NJ * VW], BF16, tag=f"v{bb}", name=f"v{bb}") for bb in range(b)]
        for bb in range(b):
            ones_cols = v_sb[bb].rearrange("p (t w) -> p t w", w=VW)[:, :, DH:VW]
            nc.vector.tensor_copy(ones_cols, ones16[:, 0:NJ].rearrange("p (t o) -> p t o", o=1))
        # oT + denominator per batch: rows 0-63 = o^T, row 64 = sum_j P~b
        ot_sb = [ot_sb_pool.tile([VW, n], F32R, tag=f"ot{bb}", name=f"ot{bb}")
                 for bb in range(b)]

        recip_sb = [const.tile([128, NJ], F32, tag=f"recip_sb{bb}", name=f"rs{bb}")
                    for bb in range(b)]

        # ---------------- emission-unit generators ----------------
        HN = min(n, 2048)   # qt chunk width (tokens per load)
        NQ = n // HN
        POOL_MULT_EVERY = 0  # every k-th ebias multiply runs on GPSIMD/Pool

        def emit_proj_batch(bb, qt_pool, pj_pool):
            """Projections for one batch; yields between PE chunks so score
            emission can interleave."""
            pair, lb = bb // 2, bb % 2
            rows = slice(64 * lb, 64 * lb + 64)
            for hh in range(NQ):
                qt_c = []
                for c in range(CC):
                    t = qt_pool.tile([128, HN], BF16, tag="qt", name="qtc")
                    nc.sync.dma_start(
                        out=t, in_=qT[c * 128:(c + 1) * 128,
                                      bb * n + hh * HN: bb * n + (hh + 1) * HN])
                    qt_c.append(t)
                for hic in range(HN // IC):
                    for wname, dest in (("wq", qT_sb[pair]), ("wk", kT_sb[pair])):
                        icc = (hh * HN + hic * IC) // IC
                        ps = pj_pool.tile([64, IC], F32, tag="pj")
                        for c in range(CC):
                            nc.tensor.matmul(
                                ps, lhsT=w_sb[wname][:, c, :],
                                rhs=qt_c[c][:, hic * IC:(hic + 1) * IC],
                                start=(c == 0), stop=(c == CC - 1))
                        nc.any.tensor_copy(dest[rows, icc * IC:(icc + 1) * IC], ps)
                        eb_feed()
                        yield
                for htt in range(0, HN // 128, 2):
                    for h2 in range(2):
                        tt = (hh * HN + (htt + h2) * 128) // 128
                        psv = pj_pool.tile([128, DH], F32, tag="pj")
                        for c in range(CC):
                            nc.tensor.matmul(
                                psv, lhsT=qt_c[c][:, (htt + h2) * 128:(htt + h2 + 1) * 128],
                                rhs=w_sb["wv"][:, c, :],
                                start=(c == 0), stop=(c == CC - 1))
                        nc.any.tensor_copy(v_sb[bb][:, tt * VW: tt * VW + DH], psv)
                    eb_feed()
                    yield

        def emit_scores_main(pair, ig, lb, box, mult_ctr=[0]):
            """Scores + softmax + attn*V for one (pair, i-group, batch);
            yields between jt steps. Leaves the live ot_ps tiles in box."""
            bb = 2 * pair + lb
            rows = slice(64 * lb, 64 * lb + 64)
            icols = slice(ig * IG, (ig + 1) * IG)
            ot_ps = box["ot_ps"] = [
                ot_pool.tile([VW, IC], F32, tag="ot", name="otp")
                for _ in range(NIL)]
            for jt in range(NJ):
                st = st_pool.tile([128, IG], F32, tag="st")
                for il in range(NIL):
                    ic = ig * NIL + il
                    nc.tensor.matmul(
                        st[:, il * IC:(il + 1) * IC],
                        lhsT=kT_sb[pair][rows, jt * 128:(jt + 1) * 128],
                        rhs=qT_sb[pair][rows, ic * IC:(ic + 1) * IC],
                        start=True, stop=True)
                pexp = p_pool.tile([128, IG], BF16, tag="pexp")
                nc.scalar.activation(
                    pexp, st, mybir.ActivationFunctionType.Exp)
                pexpb = pb_pool.tile([128, IG], BF16, tag="pexpb")
                mult_ctr[0] += 1
                meng = nc.gpsimd if (POOL_MULT_EVERY
                                     and mult_ctr[0] % POOL_MULT_EVERY == 0) else nc.vector
                meng.tensor_tensor(
                    out=pexpb, in0=pexp, in1=ebias_sb[:, jt, icols],
                    op=mybir.AluOpType.mult)
                for il in range(NIL):
                    nc.tensor.matmul(
                        ot_ps[il],
                        lhsT=v_sb[bb][:, jt * VW: jt * VW + VW],
                        rhs=pexpb[:, il * IC:(il + 1) * IC],
                        start=(jt == 0), stop=(jt == NJ - 1))
                eb_feed()
                yield

        def emit_scores_tail(pair, ig, lb, box, po_pool):
            """oT evacuation, denominator reciprocal, and output projection
            for one (pair, i-group, batch). Interleaved into the next block."""
            bb = 2 * pair + lb
            ot_ps = box["ot_ps"]
            # evacuate oT + denominator rows in one copy per chunk
            for il in range(NIL):
                ic = ig * NIL + il
                nc.vector.tensor_copy(
                    ot_sb[bb][:, ic * IC:(ic + 1) * IC], ot_ps[il])
            yield
            # denominator row -> per-token-tile columns via PE transpose-mode
            # (nearly free on the PE), then reciprocal
            den_ps = po_pool.tile([128, TPG], F32, tag="pj")
            for tg in range(TPG):
                nc.tensor.transpose(
                    den_ps[:, tg:tg + 1],
                    ot_sb[bb][64:65, (ig * TPG + tg) * 128:(ig * TPG + tg + 1) * 128],
                    ident)
            nc.vector.reciprocal(
                recip_sb[bb][:, ig * TPG:(ig + 1) * TPG], den_ps)
            yield
            TGB = min(4, TPG)  # token tiles batched per output DMA
            for tg0 in range(ig * TPG, (ig + 1) * TPG, TGB):
                osb = out_pool.tile([128, TGB, d], BF16, tag="osb")
                for tgo in range(TGB):
                    tg = tg0 + tgo
                    po = po_pool.tile([128, d], F32, tag="pj")
                    nc.tensor.matmul(
                        po, lhsT=ot_sb[bb][0:64, tg * 128:(tg + 1) * 128],
                        rhs=wout_sb[0:64, :], start=True, stop=True)
                    nc.any.tensor_scalar_mul(
                        osb[:, tgo, :], po, recip_sb[bb][:, tg: tg + 1])
                nc.sync.dma_start(
                    out=out[bb * n + tg0 * 128: bb * n + (tg0 + TGB) * 128, :]
                        .rearrange("(t p) e -> p t e", p=128),
                    in_=osb)
                if tg0 + TGB < (ig + 1) * TPG:
                    yield

        def drain(gen):
            if gen is not None:
                for _ in gen:
                    pass

        def interleave(main_gen, bg_gen, k):
            """Run main_gen to completion, pulling k items from bg_gen after
            each main item."""
            for _ in main_gen:
                if bg_gen is not None:
                    for _ in range(k):
                        if next(bg_gen, StopIteration) is StopIteration:
                            bg_gen = None
                            break
            return bg_gen

        # ---------------- emission schedule ----------------
        # proj(bb0) first; then each scores block overlaps the next batch's
        # projections on the PE while ACT/DVE chew on exp/mult.
        with tc.tile_pool(name="qt", bufs=2 * CC + 2) as qt_pool, \
             tc.tile_pool(name="pj", bufs=2, space="PSUM") as pj_pool:
            po_pool = pj_pool
            blocks = [(pair, ig, lb)
                      for pair in range(NPAIR)
                      for ig in range(NIG)
                      for lb in range(2)]
            # background projection stream: bb1, bb2, ... (bb0 emitted eagerly)
            drain(emit_proj_batch(0, qt_pool, pj_pool))
            state = {"bg": None, "bg_bb": -1, "next_bb": 1, "tail": None}

            def bg_start():
                if state["bg"] is None and state["next_bb"] < b:
                    state["bg_bb"] = state["next_bb"]
                    state["bg"] = emit_proj_batch(state["next_bb"], qt_pool, pj_pool)
                    state["next_bb"] += 1

            def ensure_projected(need_bb):
                # fully emit every projection batch <= need_bb before the
                # dependent score block enters the engine streams
                while state["bg_bb"] <= need_bb and (
                        state["bg"] is not None or state["next_bb"] <= need_bb):
                    bg_start()
                    drain(state["bg"])
                    state["bg"] = None
                    if state["next_bb"] <= need_bb:
                        continue
                    break

            def feed():
                # one unit from the previous block's tail, then up to two
                # units from the background projection stream
                if state["tail"] is not None:
                    if next(state["tail"], StopIteration) is StopIteration:
                        state["tail"] = None
                for _ in range(2):
                    bg_start()
                    if state["bg"] is None:
                        break
                    if next(state["bg"], StopIteration) is StopIteration:
                        state["bg"] = None

            for pair, ig, lb in blocks:
                ensure_projected(2 * pair + lb)
                box = {}
                for _ in emit_scores_main(pair, ig, lb, box):
                    feed()
                drain(state["tail"])
                state["tail"] = emit_scores_tail(pair, ig, lb, box, po_pool)
            drain(state["tail"])
            drain(state["bg"])
            state["bg"] = None
            while state["next_bb"] < b:
                bg_start()
                drain(state["bg"])
                state["bg"] = None
    nc.compile()
    return nc


def make_in_maps(query, pos_bias, Wq, Wk, Wv, Wout, n_cores=N_CORES):
    """Host-side sharding/layout prep. Head h -> core h."""
    query = np.asarray(query, dtype=np.float32)
    pos_bias = np.asarray(pos_bias, dtype=np.float32)
    Wq = np.asarray(Wq, dtype=np.float32)
    Wk = np.asarray(Wk, dtype=np.float32)
    Wv = np.asarray(Wv, dtype=np.float32)
    Wout = np.asarray(Wout, dtype=np.float32)

    b, n, d = query.shape
    qT = np.ascontiguousarray(query.reshape(b * n, d).T).astype(BF)
    wq_s = (Wq * np.float32(SCALE)).astype(BF)
    wk_b = Wk.astype(BF)
    wv_b = Wv.astype(BF)
    in_maps = []
    for h in range(n_cores):
        sl = slice(h * DH, (h + 1) * DH)
        in_maps.append({
            "qT": qT,
            "ebiasT": np.ascontiguousarray(np.exp(pos_bias[h]).T.astype(BF)),
            "wq": np.ascontiguousarray(wq_s[:, sl]),
            "wk": np.ascontiguousarray(wk_b[:, sl]),
            "wv": np.ascontiguousarray(wv_b[:, sl]),
            "wout": np.ascontiguousarray(Wout[sl, :]),
        })
    return in_maps


def run_device(in_maps, b=B, n=N, d=D, packed=False, trace=False, **kw):
    nc = build_nc(b, n, d, packed, n_cores=len(in_maps))
    return run_bass_kernel_spmd(nc, in_maps, list(range(len(in_maps))), trace=trace, **kw)


def assemble(results, b=B, n=N, d=D):
    acc = np.zeros((b * n, d), dtype=np.float32)
    for r in results:
        acc += r["out"].astype(np.float32)
    return acc.reshape(b, n, d)


def kernel(query, pos_bias, Wq, Wk, Wv, Wout):
    in_maps = make_in_maps(query, pos_bias, Wq, Wk, Wv, Wout)
    res = run_device(in_maps)
    return assemble(res.results)


# revision 35
# speedup vs baseline: 1.6741x; 1.0633x over previous
"""Multi-head self-attention with positional bias, sharded over 8 NeuronCores.

Sharding: head-parallel. Core h computes head h for all batches:
  q/k/v projections with the head's weight slices, scores + softmax with the
  head's pos_bias slice, and the partial output  o_h @ Wout[h*64:(h+1)*64, :].
The full output is the sum of the 8 partials (row-parallel Wout).

Device kernel math (per core):
  - query is supplied pre-transposed (qT [D, B*N], bf16) so the contraction
    dim of every projection lands on SBUF partitions.
  - scores are computed TRANSPOSED: ST[j, i] = k_j . q_i (scaled), so exp's
    output P~[j, i] is directly the layout the attention*V matmul needs.
  - the positional bias is folded in multiplicatively AFTER exp:
      exp(qk + bias) = exp(qk) * exp(bias)
    with exp(bias) precomputed on the host (bf16, SBUF-resident for the whole
    kernel). This removes the PE-side bias injection entirely; the multiply
    runs on the DVE in bf16 (2x perf mode).
  - exp skips max-subtraction: qk scores are ~N(0, 1), exp is safe in fp32/bf16
    and softmax is shift-invariant.
  - softmax denominator: a ones column appended to v, so attention*V's 65th
    output row is sum_j P~b[j, i]; normalization is deferred to the PSUM
    evacuation after the Wout matmul (per-partition scalar multiply).
  - projections and attention*V run in bf16 (1 cyc/row on PE at any free dim,
    fp32 PSUM accumulation); scores qk runs in float32r (full q/k precision).
  - PSUM is statically partitioned (proj 2 banks -> reused by output proj,
    scores 4, attn*V accum 2) so the score pipeline overlaps the projections
    and the per-(pair, ig) output projections overlap later score groups.
"""

import numpy as np
from contextlib import ExitStack

import ml_dtypes

import concourse.bass as bass
import concourse.bacc as bacc
import concourse.mybir as mybir
import concourse.tile as tile
from concourse.bass_utils import run_bass_kernel_spmd
from concourse.masks import make_identity

HEADS = 8
DH = 64
B, N, D = 4, 2048, 512
SCALE = DH ** -0.5
N_CORES = 8
PACKED = False  # unused; kept for test.py compatibility

F32 = mybir.dt.float32
F32R = mybir.dt.float32r
BF16 = mybir.dt.bfloat16
BF = ml_dtypes.bfloat16


def build_nc(b=B, n=N, d=D, packed=False, n_cores=1):
    """Build the per-core Bass program. All cores run the same program (SPMD);
    per-head differences come in through the input tensors."""
    assert b % 2 == 0 and n % 512 == 0 and d % 128 == 0
    T = b * n           # total tokens
    CC = d // 128       # contraction chunks for the projections
    NJ = n // 128       # key tiles (j)
    IC = 512            # matmul output column chunk (one PSUM bank)
    IG = min(n, 1024)   # exp/mult group width (2 PSUM banks)
    NIG = n // IG
    NIL = IG // IC      # IC chunks per group
    NPAIR = b // 2
    VW = DH + 2         # v block width (+2 ones columns for the denominator;
                        # 2 so the PE transpose has an even output count)
    TPG = IG // 128     # token tiles per i-group

    nc = bacc.Bacc("TRN2", target_bir_lowering=False, debug=False,
                   num_devices=n_cores)
    qT = nc.declare_dram_parameter("qT", [d, T], BF16, isOutput=False)
    ebiasT = nc.declare_dram_parameter("ebiasT", [n, n], BF16, isOutput=False)
    wq = nc.declare_dram_parameter("wq", [d, DH], BF16, isOutput=False)
    wk = nc.declare_dram_parameter("wk", [d, DH], BF16, isOutput=False)
    wv = nc.declare_dram_parameter("wv", [d, DH], BF16, isOutput=False)
    wout = nc.declare_dram_parameter("wout", [DH, d], F32R, isOutput=False)
    out = nc.declare_dram_parameter("out", [T, d], BF16, isOutput=True)

    with ExitStack() as ctx:
        tc = ctx.enter_context(tile.TileContext(nc))

        const = ctx.enter_context(tc.tile_pool(name="const", bufs=1))
        eb_pool = ctx.enter_context(tc.tile_pool(name="ebias", bufs=1))
        qk_pool = ctx.enter_context(tc.tile_pool(name="qkT", bufs=1))
        v_pool = ctx.enter_context(tc.tile_pool(name="v", bufs=1))
        ot_sb_pool = ctx.enter_context(tc.tile_pool(name="ot_sb", bufs=1))
        p_pool = ctx.enter_context(tc.tile_pool(name="pexp", bufs=6))
        pb_pool = ctx.enter_context(tc.tile_pool(name="pexpb", bufs=6))
        out_pool = ctx.enter_context(tc.tile_pool(name="osb", bufs=2))
        # PSUM: st (4 banks) + ot (2) stay open for the whole kernel; the
        # projection pool (2 banks) closes before the output-projection pool
        # (2 banks) opens, so the static footprint never exceeds 8 banks.
        st_pool = ctx.enter_context(tc.tile_pool(name="st", bufs=2, space="PSUM"))
        ot_pool = ctx.enter_context(tc.tile_pool(name="ot", bufs=2, space="PSUM"))

        ones32 = const.tile([128, 2 * 16], BF16, tag="ones32")
        nc.vector.memset(ones32, 1.0)
        ident_f32 = const.tile([128, 128], F32, tag="ident_f32")
        make_identity(nc, ident_f32)
        ident = const.tile([128, 128], F32R, tag="ident")
        nc.vector.tensor_copy(ident, ident_f32)

        w_sb = {}
        for name, w in (("wq", wq), ("wk", wk), ("wv", wv)):
            t = const.tile([128, CC, DH], BF16, tag=name)
            nc.sync.dma_start(out=t, in_=w[:, :].rearrange("(c p) e -> p c e", p=128))
            w_sb[name] = t
        wout_sb = const.tile([64, d], F32R, tag="wout")
        nc.sync.dma_start(out=wout_sb[0:64, :], in_=wout[:, :])

        # full exp(bias) head-slice, resident in SBUF: [128, NJ, n] bf16.
        # Loads are staggered through the early emission (see eb_feed) so they
        # don't monopolize the DMA engines ahead of the qt loads.
        ebias_sb = eb_pool.tile([128, NJ, n], BF16, tag="ebias")

        def _eb_load(jt, ig):
            nc.sync.dma_start(
                out=ebias_sb[:, jt, ig * IG:(ig + 1) * IG],
                in_=ebiasT[jt * 128:(jt + 1) * 128, ig * IG:(ig + 1) * IG])
        eb_pending = [(jt, ig) for ig in range(NIG) for jt in range(NJ)]

        def eb_feed(k=1):
            for _ in range(k):
                if eb_pending:
                    _eb_load(*eb_pending.pop(0))

        qT_sb = [qk_pool.tile([128, n], F32R, tag=f"qT{p}", name=f"qT{p}") for p in range(NPAIR)]
        kT_sb = [qk_pool.tile([128, n], F32R, tag=f"kT{p}", name=f"kT{p}") for p in range(NPAIR)]
        v_sb = [v_pool.tile([128, NJ * VW], BF16, tag=f"v{bb}", name=f"v{bb}") for bb in range(b)]
        for bb in range(b):
            ones_cols = v_sb[bb].rearrange("p (t w) -> p t w", w=VW)[:, :, DH:VW]
            nc.vector.tensor_copy(
                ones_cols, ones32[:, 0:2 * NJ].rearrange("p (t o) -> p t o", o=2))
        # oT + denominator per batch: rows 0-63 = o^T, rows 64-65 = sum_j P~b
        ot_sb = [ot_sb_pool.tile([VW, n], F32R, tag=f"ot{bb}", name=f"ot{bb}")
                 for bb in range(b)]

        recip_sb = [const.tile([128, NJ], F32, tag=f"recip_sb{bb}", name=f"rs{bb}")
                    for bb in range(b)]

        # ---------------- emission-unit generators ----------------
        HN = min(n, 2048)   # qt chunk width (tokens per load)
        NQ = n // HN
        POOL_MULT_EVERY = 0  # every k-th ebias multiply runs on GPSIMD/Pool

        def emit_proj_batch(bb, qt_pool, pj_pool):
            """Projections for one batch; yields between PE chunks so score
            emission can interleave."""
            pair, lb = bb // 2, bb % 2
            rows = slice(64 * lb, 64 * lb + 64)
            ceng = nc.scalar if bb == 0 else nc.vector
            for hh in range(NQ):
                qt_c = [qt_pool.tile([128, HN], BF16, tag="qt", name="qtc")
                        for _ in range(CC)]
                base = bb * n + hh * HN
                for s in range(0, HN, IC):
                    for c in range(CC):
                        nc.sync.dma_start(
                            out=qt_c[c][:, s:s + IC],
                            in_=qT[c * 128:(c + 1) * 128, base + s: base + s + IC])
                NH = HN // IC
                qk_order = [("wq", 0), ("wk", 0), ("wq", 1), ("wk", 1)] + \
                    [("wk", h) for h in range(2, NH)] + \
                    [("wq", h) for h in range(2, NH)]
                for wname, hic in qk_order:
                    if True:
                        dest = qT_sb[pair] if wname == "wq" else kT_sb[pair]
                        icc = (hh * HN + hic * IC) // IC
                        ps = pj_pool.tile([64, IC], F32, tag="pj")
                        for c in range(CC):
                            nc.tensor.matmul(
                                ps, lhsT=w_sb[wname][:, c, :],
                                rhs=qt_c[c][:, hic * IC:(hic + 1) * IC],
                                start=(c == 0), stop=(c == CC - 1))
                        ceng.copy(dest[rows, icc * IC:(icc + 1) * IC], ps) if bb == 0 else nc.vector.tensor_copy(dest[rows, icc * IC:(icc + 1) * IC], ps)
                        eb_feed()
                        yield
                v_view = v_sb[bb].rearrange("p (t w) -> p t w", w=VW)
                for htt in range(0, HN // 128, 2):
                    psv = pj_pool.tile([128, 2, DH], F32, tag="pj")
                    for h2 in range(2):
                        for c in range(CC):
                            nc.tensor.matmul(
                                psv[:, h2, :],
                                lhsT=qt_c[c][:, (htt + h2) * 128:(htt + h2 + 1) * 128],
                                rhs=w_sb["wv"][:, c, :],
                                start=(c == 0), stop=(c == CC - 1))
                    tt = (hh * HN + htt * 128) // 128
                    ceng.copy(v_view[:, tt:tt + 2, 0:DH], psv) if bb == 0 else nc.vector.tensor_copy(v_view[:, tt:tt + 2, 0:DH], psv)
                    eb_feed()
                    yield

        def emit_scores_main(pair, ig, lb, box, mult_ctr=[0]):
            """Scores + softmax + attn*V for one (pair, i-group, batch);
            yields between jt steps. Leaves the live ot_ps tiles in box."""
            bb = 2 * pair + lb
            rows = slice(64 * lb, 64 * lb + 64)
            icols = slice(ig * IG, (ig + 1) * IG)
            ot_ps = box["ot_ps"] = [
                ot_pool.tile([VW, IC], F32, tag="ot", name="otp")
                for _ in range(NIL)]
            for jt in range(NJ):
                box["jt"] = jt
                st = st_pool.tile([128, IG], F32, tag="st")
                for il in range(NIL):
                    ic = ig * NIL + il
                    nc.tensor.matmul(
                        st[:, il * IC:(il + 1) * IC],
                        lhsT=kT_sb[pair][rows, jt * 128:(jt + 1) * 128],
                        rhs=qT_sb[pair][rows, ic * IC:(ic + 1) * IC],
                        start=True, stop=True)
                pexp = p_pool.tile([128, IG], BF16, tag="pexp")
                nc.scalar.activation(
                    pexp, st, mybir.ActivationFunctionType.Exp)
                pexpb = pb_pool.tile([128, IG], BF16, tag="pexpb")
                mult_ctr[0] += 1
                meng = nc.gpsimd if (POOL_MULT_EVERY
                                     and mult_ctr[0] % POOL_MULT_EVERY == 0) else nc.vector
                meng.tensor_tensor(
                    out=pexpb, in0=pexp, in1=ebias_sb[:, jt, icols],
                    op=mybir.AluOpType.mult)
                for il in range(NIL):
                    nc.tensor.matmul(
                        ot_ps[il],
                        lhsT=v_sb[bb][:, jt * VW: jt * VW + VW],
                        rhs=pexpb[:, il * IC:(il + 1) * IC],
                        start=(jt == 0), stop=(jt == NJ - 1))
                eb_feed()
                yield

        def emit_scores_tail(pair, ig, lb, box, po_pool, is_last=False):
            """oT evacuation, denominator reciprocal, and output projection
            for one (pair, i-group, batch). Interleaved into the next block."""
            bb = 2 * pair + lb
            ot_ps = box["ot_ps"]
            # evacuate oT + denominator rows in one copy per chunk
            for il in range(NIL):
                ic = ig * NIL + il
                nc.vector.tensor_copy(
                    ot_sb[bb][:, ic * IC:(ic + 1) * IC], ot_ps[il])
            yield
            # denominator row -> per-token-tile columns via PE transpose-mode
            # (nearly free on the PE), then reciprocal
            den_ps = po_pool.tile([128, 2 * TPG], F32R, tag="pj")
            for tg in range(TPG):
                nc.tensor.transpose(
                    den_ps[:, 2 * tg:2 * tg + 2],
                    ot_sb[bb][64:66, (ig * TPG + tg) * 128:(ig * TPG + tg + 1) * 128],
                    ident[64:66, 64:66])
            nc.vector.reciprocal(
                recip_sb[bb][:, ig * TPG:(ig + 1) * TPG],
                den_ps.rearrange("p (t o) -> p t o", o=2)[:, :, 0])
            yield
            TGB = min(4, TPG)  # token tiles batched per output DMA
            for tg0 in range(ig * TPG, (ig + 1) * TPG, TGB):
                osb = out_pool.tile([128, TGB, d], BF16, tag="osb")
                for tgo in range(TGB):
                    tg = tg0 + tgo
                    po = po_pool.tile([128, d], F32, tag="pj")
                    nc.tensor.matmul(
                        po, lhsT=ot_sb[bb][0:64, tg * 128:(tg + 1) * 128],
                        rhs=wout_sb[0:64, :], start=True, stop=True)
                    nc.any.tensor_scalar_mul(
                        osb[:, tgo, :], po, recip_sb[bb][:, tg: tg + 1])
                nc.sync.dma_start(
                    out=out[bb * n + tg0 * 128: bb * n + (tg0 + TGB) * 128, :]
                        .rearrange("(t p) e -> p t e", p=128),
                    in_=osb)
                if tg0 + TGB < (ig + 1) * TPG:
                    yield

        def drain(gen):
            if gen is not None:
                for _ in gen:
                    pass

        def interleave(main_gen, bg_gen, k):
            """Run main_gen to completion, pulling k items from bg_gen after
            each main item."""
            for _ in main_gen:
                if bg_gen is not None:
                    for _ in range(k):
                        if next(bg_gen, StopIteration) is StopIteration:
                            bg_gen = None
                            break
            return bg_gen

        # ---------------- emission schedule ----------------
        # proj(bb0) first; then each scores block overlaps the next batch's
        # projections on the PE while ACT/DVE chew on exp/mult.
        with tc.tile_pool(name="qt", bufs=CC + 4) as qt_pool, \
             tc.tile_pool(name="pj", bufs=2, space="PSUM") as pj_pool:
            po_pool = pj_pool
            blocks = [(pair, ig, lb)
                      for pair in range(NPAIR)
                      for lb in range(2)
                      for ig in range(NIG)]
            # background projection stream: bb1, bb2, ... (bb0 emitted eagerly)
            drain(emit_proj_batch(0, qt_pool, pj_pool))
            state = {"bg": None, "bg_bb": -1, "next_bb": 1, "tail": None}

            def bg_start():
                if state["bg"] is None and state["next_bb"] < b:
                    state["bg_bb"] = state["next_bb"]
                    state["bg"] = emit_proj_batch(state["next_bb"], qt_pool, pj_pool)
                    state["next_bb"] += 1

            def ensure_projected(need_bb):
                # fully emit every projection batch <= need_bb before the
                # dependent score block enters the engine streams
                while state["bg_bb"] <= need_bb and (
                        state["bg"] is not None or state["next_bb"] <= need_bb):
                    bg_start()
                    drain(state["bg"])
                    state["bg"] = None
                    if state["next_bb"] <= need_bb:
                        continue
                    break

            def feed(jt, blk):
                # previous block's tail: evac/recip units right away (they
                # free the ot accumulators), but hold the output-projection
                # units until the reciprocal had time to land -- otherwise
                # its PSUM matmuls clog the PE wait queue while blocked on
                # recip, starving the new block's qk matmuls.
                if state["tail"] is not None and (jt < 2 or jt >= 5):
                    if next(state["tail"], StopIteration) is StopIteration:
                        state["tail"] = None
                # trickle the background projections: the PE only has
                # ~0.2us/jt of slack during a score block, so one unit per
                # jt in the first block (bb1 must finish in time) and one
                # every other jt after that
                if blk == 0 or jt % 2 == 0:
                    bg_start()
                    if state["bg"] is not None:
                        if next(state["bg"], StopIteration) is StopIteration:
                            state["bg"] = None

            for blk, (pair, ig, lb) in enumerate(blocks):
                ensure_projected(2 * pair + lb)
                box = {}
                for _ in emit_scores_main(pair, ig, lb, box):
                    feed(box["jt"], blk)
                drain(state["tail"])
                state["tail"] = emit_scores_tail(
                    pair, ig, lb, box, po_pool,
                    is_last=(pair, ig, lb) == blocks[-1])
            drain(state["tail"])
            drain(state["bg"])
            state["bg"] = None
            while state["next_bb"] < b:
                bg_start()
                drain(state["bg"])
                state["bg"] = None
    nc.compile()
    return nc


def make_in_maps(query, pos_bias, Wq, Wk, Wv, Wout, n_cores=N_CORES):
    """Host-side sharding/layout prep. Head h -> core h."""
    query = np.asarray(query, dtype=np.float32)
    pos_bias = np.asarray(pos_bias, dtype=np.float32)
    Wq = np.asarray(Wq, dtype=np.float32)
    Wk = np.asarray(Wk, dtype=np.float32)
    Wv = np.asarray(Wv, dtype=np.float32)
    Wout = np.asarray(Wout, dtype=np.float32)

    b, n, d = query.shape
    qT = np.ascontiguousarray(query.reshape(b * n, d).T).astype(BF)
    wq_s = (Wq * np.float32(SCALE)).astype(BF)
    wk_b = Wk.astype(BF)
    wv_b = Wv.astype(BF)
    in_maps = []
    for h in range(n_cores):
        sl = slice(h * DH, (h + 1) * DH)
        in_maps.append({
            "qT": qT,
            "ebiasT": np.ascontiguousarray(np.exp(pos_bias[h]).T.astype(BF)),
            "wq": np.ascontiguousarray(wq_s[:, sl]),
            "wk": np.ascontiguousarray(wk_b[:, sl]),
            "wv": np.ascontiguousarray(wv_b[:, sl]),
            "wout": np.ascontiguousarray(Wout[sl, :]),
        })
    return in_maps


def run_device(in_maps, b=B, n=N, d=D, packed=False, trace=False, **kw):
    nc = build_nc(b, n, d, packed, n_cores=len(in_maps))
    return run_bass_kernel_spmd(nc, in_maps, list(range(len(in_maps))), trace=trace, **kw)


def assemble(results, b=B, n=N, d=D):
    acc = np.zeros((b * n, d), dtype=np.float32)
    for r in results:
        acc += r["out"].astype(np.float32)
    return acc.reshape(b, n, d)


def kernel(query, pos_bias, Wq, Wk, Wv, Wout):
    in_maps = make_in_maps(query, pos_bias, Wq, Wk, Wv, Wout)
    res = run_device(in_maps)
    return assemble(res.results)


# revision 45
# speedup vs baseline: 1.6815x; 1.0044x over previous
"""Multi-head self-attention with positional bias, sharded over 8 NeuronCores.

Sharding: head-parallel. Core h computes head h for all batches:
  q/k/v projections with the head's weight slices, scores + softmax with the
  head's pos_bias slice, and the partial output  o_h @ Wout[h*64:(h+1)*64, :].
The full output is the sum of the 8 partials (row-parallel Wout).

Device kernel math (per core):
  - query is supplied pre-transposed (qT [D, B*N], bf16) so the contraction
    dim of every projection lands on SBUF partitions.
  - scores are computed TRANSPOSED: ST[j, i] = k_j . q_i (scaled), so exp's
    output P~[j, i] is directly the layout the attention*V matmul needs.
  - the positional bias is folded in multiplicatively AFTER exp:
      exp(qk + bias) = exp(qk) * exp(bias)
    with exp(bias) precomputed on the host (bf16, SBUF-resident for the whole
    kernel). This removes the PE-side bias injection entirely; the multiply
    runs on the DVE in bf16 (2x perf mode).
  - exp skips max-subtraction: qk scores are ~N(0, 1), exp is safe in fp32/bf16
    and softmax is shift-invariant.
  - softmax denominator: two ones columns appended to v, so attention*V's
    output rows 64/65 are sum_j P~b[j, i]; the row is moved to per-token
    partitions with (nearly free) PE transpose-mode matmuls ([2,128]->[128,2],
    even counts to satisfy the fp32r ISA restriction), and normalization is
    deferred to the PSUM evacuation after the Wout matmul (per-partition
    scalar multiply with the reciprocal).
  - projections and attention*V run in bf16 (1 cyc/row on PE at any free dim,
    fp32 PSUM accumulation); scores qk runs in float32r (full q/k precision);
    the output partials are written back in bf16 and summed on the host.
  - PSUM is statically partitioned: scores 4 banks + attn*V accumulators 2 +
    a shared 2-bank ring for projection/output-projection/denominator tiles,
    so the score pipeline overlaps the projections and each block's output
    projection overlaps the next block's scores.
  - emission is software-pipelined: per-batch projection units and the
    previous block's evacuation/output-projection units are interleaved into
    the 16-step jt loop at rates tuned so the PE and the DMA queue (ebias and
    qt loads) stay just ahead of the ACT exp cadence, which is the critical
    resource (~133us of exp at 0.83 ns/element/partition).
"""

import numpy as np
from contextlib import ExitStack

import ml_dtypes

import concourse.bass as bass
import concourse.bacc as bacc
import concourse.mybir as mybir
import concourse.tile as tile
from concourse.bass_utils import run_bass_kernel_spmd
from concourse.masks import make_identity

HEADS = 8
DH = 64
B, N, D = 4, 2048, 512
SCALE = DH ** -0.5
N_CORES = 8
PACKED = False  # unused; kept for test.py compatibility

F32 = mybir.dt.float32
F32R = mybir.dt.float32r
BF16 = mybir.dt.bfloat16
BF = ml_dtypes.bfloat16


def build_nc(b=B, n=N, d=D, packed=False, n_cores=1):
    """Build the per-core Bass program. All cores run the same program (SPMD);
    per-head differences come in through the input tensors."""
    assert b % 2 == 0 and n % 512 == 0 and d % 128 == 0
    T = b * n           # total tokens
    CC = d // 128       # contraction chunks for the projections
    NJ = n // 128       # key tiles (j)
    IC = 512            # matmul output column chunk (one PSUM bank)
    IG = min(n, 1024)   # exp/mult group width (2 PSUM banks)
    NIG = n // IG
    NIL = IG // IC      # IC chunks per group
    NPAIR = b // 2
    VW = DH + 2         # v block width (+2 ones columns for the denominator;
                        # 2 so the PE transpose has an even output count)
    TPG = IG // 128     # token tiles per i-group

    nc = bacc.Bacc("TRN2", target_bir_lowering=False, debug=False,
                   num_devices=n_cores)
    qT = nc.declare_dram_parameter("qT", [d, T], BF16, isOutput=False)
    ebiasT = nc.declare_dram_parameter("ebiasT", [n, n], BF16, isOutput=False)
    wq = nc.declare_dram_parameter("wq", [d, DH], BF16, isOutput=False)
    wk = nc.declare_dram_parameter("wk", [d, DH], BF16, isOutput=False)
    wv = nc.declare_dram_parameter("wv", [d, DH], BF16, isOutput=False)
    wout = nc.declare_dram_parameter("wout", [DH, d], F32R, isOutput=False)
    out = nc.declare_dram_parameter("out", [T, d], BF16, isOutput=True)

    with ExitStack() as ctx:
        tc = ctx.enter_context(tile.TileContext(nc))

        const = ctx.enter_context(tc.tile_pool(name="const", bufs=1))
        eb_pool = ctx.enter_context(tc.tile_pool(name="ebias", bufs=1))
        qk_pool = ctx.enter_context(tc.tile_pool(name="qkT", bufs=1))
        v_pool = ctx.enter_context(tc.tile_pool(name="v", bufs=1))
        ot_sb_pool = ctx.enter_context(tc.tile_pool(name="ot_sb", bufs=1))
        p_pool = ctx.enter_context(tc.tile_pool(name="pexp", bufs=6))
        pb_pool = ctx.enter_context(tc.tile_pool(name="pexpb", bufs=6))
        out_pool = ctx.enter_context(tc.tile_pool(name="osb", bufs=2))
        # PSUM: st (4 banks) + ot (2) stay open for the whole kernel; the
        # projection pool (2 banks) closes before the output-projection pool
        # (2 banks) opens, so the static footprint never exceeds 8 banks.
        st_pool = ctx.enter_context(tc.tile_pool(name="st", bufs=2, space="PSUM"))
        ot_pool = ctx.enter_context(tc.tile_pool(name="ot", bufs=2, space="PSUM"))

        ones32 = const.tile([128, 2 * 16], BF16, tag="ones32")
        nc.vector.memset(ones32, 1.0)
        ident_f32 = const.tile([128, 128], F32, tag="ident_f32")
        make_identity(nc, ident_f32)
        ident = const.tile([128, 128], F32R, tag="ident")
        nc.vector.tensor_copy(ident, ident_f32)

        w_sb = {}
        for name, w in (("wq", wq), ("wk", wk), ("wv", wv)):
            t = const.tile([128, CC, DH], BF16, tag=name)
            nc.sync.dma_start(out=t, in_=w[:, :].rearrange("(c p) e -> p c e", p=128))
            w_sb[name] = t
        wout_sb = const.tile([64, d], F32R, tag="wout")
        nc.sync.dma_start(out=wout_sb[0:64, :], in_=wout[:, :])

        # full exp(bias) head-slice, resident in SBUF: [128, NJ, n] bf16.
        # Loads are staggered through the early emission (see eb_feed) so they
        # don't monopolize the DMA engines ahead of the qt loads.
        ebias_sb = eb_pool.tile([128, NJ, n], BF16, tag="ebias")

        def _eb_load(jt, ig):
            nc.sync.dma_start(
                out=ebias_sb[:, jt, ig * IG:(ig + 1) * IG],
                in_=ebiasT[jt * 128:(jt + 1) * 128, ig * IG:(ig + 1) * IG])
        eb_pending = [(jt, ig) for ig in range(NIG) for jt in range(NJ)]

        def eb_feed(k=1):
            for _ in range(k):
                if eb_pending:
                    _eb_load(*eb_pending.pop(0))

        qT_sb = [qk_pool.tile([128, n], F32R, tag=f"qT{p}", name=f"qT{p}") for p in range(NPAIR)]
        kT_sb = [qk_pool.tile([128, n], F32R, tag=f"kT{p}", name=f"kT{p}") for p in range(NPAIR)]
        v_sb = [v_pool.tile([128, NJ * VW], BF16, tag=f"v{bb}", name=f"v{bb}") for bb in range(b)]
        for bb in range(b):
            ones_cols = v_sb[bb].rearrange("p (t w) -> p t w", w=VW)[:, :, DH:VW]
            nc.vector.tensor_copy(
                ones_cols, ones32[:, 0:2 * NJ].rearrange("p (t o) -> p t o", o=2))
        # oT + denominator per batch: rows 0-63 = o^T, rows 64-65 = sum_j P~b
        ot_sb = [ot_sb_pool.tile([VW, n], F32R, tag=f"ot{bb}", name=f"ot{bb}")
                 for bb in range(b)]

        recip_sb = [const.tile([128, NJ], F32, tag=f"recip_sb{bb}", name=f"rs{bb}")
                    for bb in range(b)]

        # ---------------- emission-unit generators ----------------
        HN = min(n, 2048)   # qt chunk width (tokens per load)
        NQ = n // HN
        POOL_MULT_EVERY = 0  # every k-th ebias multiply runs on GPSIMD/Pool

        def emit_proj_batch(bb, qt_pool, pj_pool):
            """Projections for one batch; yields between PE chunks so score
            emission can interleave."""
            pair, lb = bb // 2, bb % 2
            rows = slice(64 * lb, 64 * lb + 64)
            for hh in range(NQ):
                qt_c = [qt_pool.tile([128, HN], BF16, tag="qt", name="qtc")
                        for _ in range(CC)]
                base = bb * n + hh * HN
                loaded = set()

                def qt_load(hic):
                    # lazy per-chunk loads keep this batch's qt DMAs from
                    # jumping the queue ahead of more urgent ebias tiles
                    if hic in loaded:
                        return
                    loaded.add(hic)
                    s = hic * IC
                    for c in range(CC):
                        nc.sync.dma_start(
                            out=qt_c[c][:, s:s + IC],
                            in_=qT[c * 128:(c + 1) * 128, base + s: base + s + IC])
                NH = HN // IC
                qk_order = [("wq", 0), ("wk", 0), ("wq", 1), ("wk", 1)] + \
                    [("wk", h) for h in range(2, NH)] + \
                    [("wq", h) for h in range(2, NH)]
                for wname, hic in qk_order:
                    qt_load(hic)
                    dest = qT_sb[pair] if wname == "wq" else kT_sb[pair]
                    icc = (hh * HN + hic * IC) // IC
                    ps = pj_pool.tile([64, IC], F32, tag="pj")
                    for c in range(CC):
                        nc.tensor.matmul(
                            ps, lhsT=w_sb[wname][:, c, :],
                            rhs=qt_c[c][:, hic * IC:(hic + 1) * IC],
                            start=(c == 0), stop=(c == CC - 1))
                    if bb == 0:  # ACT is idle before the first exp
                        nc.scalar.copy(dest[rows, icc * IC:(icc + 1) * IC], ps)
                    else:
                        nc.vector.tensor_copy(dest[rows, icc * IC:(icc + 1) * IC], ps)
                    eb_feed()
                    yield
                v_view = v_sb[bb].rearrange("p (t w) -> p t w", w=VW)
                for htt in range(0, HN // 128, 2):
                    psv = pj_pool.tile([128, 2, DH], F32, tag="pj")
                    for h2 in range(2):
                        for c in range(CC):
                            nc.tensor.matmul(
                                psv[:, h2, :],
                                lhsT=qt_c[c][:, (htt + h2) * 128:(htt + h2 + 1) * 128],
                                rhs=w_sb["wv"][:, c, :],
                                start=(c == 0), stop=(c == CC - 1))
                    tt = (hh * HN + htt * 128) // 128
                    if bb == 0:
                        nc.scalar.copy(v_view[:, tt:tt + 2, 0:DH], psv)
                    else:
                        nc.vector.tensor_copy(v_view[:, tt:tt + 2, 0:DH], psv)
                    eb_feed()
                    yield

        def emit_scores_main(pair, ig, lb, box, mult_ctr=[0]):
            """Scores + softmax + attn*V for one (pair, i-group, batch);
            yields between jt steps. Leaves the live ot_ps tiles in box."""
            bb = 2 * pair + lb
            rows = slice(64 * lb, 64 * lb + 64)
            icols = slice(ig * IG, (ig + 1) * IG)
            ot_ps = box["ot_ps"] = [
                ot_pool.tile([VW, IC], F32, tag="ot", name="otp")
                for _ in range(NIL)]
            for jt in range(NJ):
                box["jt"] = jt
                st = st_pool.tile([128, IG], F32, tag="st")
                for il in range(NIL):
                    ic = ig * NIL + il
                    nc.tensor.matmul(
                        st[:, il * IC:(il + 1) * IC],
                        lhsT=kT_sb[pair][rows, jt * 128:(jt + 1) * 128],
                        rhs=qT_sb[pair][rows, ic * IC:(ic + 1) * IC],
                        start=True, stop=True)
                pexp = p_pool.tile([128, IG], BF16, tag="pexp")
                nc.scalar.activation(
                    pexp, st, mybir.ActivationFunctionType.Exp)
                pexpb = pb_pool.tile([128, IG], BF16, tag="pexpb")
                mult_ctr[0] += 1
                meng = nc.gpsimd if (POOL_MULT_EVERY
                                     and mult_ctr[0] % POOL_MULT_EVERY == 0) else nc.vector
                meng.tensor_tensor(
                    out=pexpb, in0=pexp, in1=ebias_sb[:, jt, icols],
                    op=mybir.AluOpType.mult)
                for il in range(NIL):
                    nc.tensor.matmul(
                        ot_ps[il],
                        lhsT=v_sb[bb][:, jt * VW: jt * VW + VW],
                        rhs=pexpb[:, il * IC:(il + 1) * IC],
                        start=(jt == 0), stop=(jt == NJ - 1))
                eb_feed(2)
                yield

        def emit_scores_tail(pair, ig, lb, box, po_pool, is_last=False):
            """oT evacuation, denominator reciprocal, and output projection
            for one (pair, i-group, batch). Interleaved into the next block."""
            bb = 2 * pair + lb
            ot_ps = box["ot_ps"]
            # evacuate oT + denominator rows in one copy per chunk
            for il in range(NIL):
                ic = ig * NIL + il
                nc.vector.tensor_copy(
                    ot_sb[bb][:, ic * IC:(ic + 1) * IC], ot_ps[il])
            yield
            # denominator row -> per-token-tile columns via PE transpose-mode
            # (nearly free on the PE), then reciprocal
            den_ps = po_pool.tile([128, 2 * TPG], F32R, tag="pj")
            for tg in range(TPG):
                nc.tensor.transpose(
                    den_ps[:, 2 * tg:2 * tg + 2],
                    ot_sb[bb][64:66, (ig * TPG + tg) * 128:(ig * TPG + tg + 1) * 128],
                    ident[64:66, 64:66])
            nc.vector.reciprocal(
                recip_sb[bb][:, ig * TPG:(ig + 1) * TPG],
                den_ps.rearrange("p (t o) -> p t o", o=2)[:, :, 0])
            yield
            TGB = min(4, TPG)  # token tiles batched per output DMA
            for tg0 in range(ig * TPG, (ig + 1) * TPG, TGB):
                osb = out_pool.tile([128, TGB, d], BF16, tag="osb")
                for tgo in range(TGB):
                    tg = tg0 + tgo
                    po = po_pool.tile([128, d], F32, tag="pj")
                    nc.tensor.matmul(
                        po, lhsT=ot_sb[bb][0:64, tg * 128:(tg + 1) * 128],
                        rhs=wout_sb[0:64, :], start=True, stop=True)
                    nc.any.tensor_scalar_mul(
                        osb[:, tgo, :], po, recip_sb[bb][:, tg: tg + 1])
                nc.sync.dma_start(
                    out=out[bb * n + tg0 * 128: bb * n + (tg0 + TGB) * 128, :]
                        .rearrange("(t p) e -> p t e", p=128),
                    in_=osb)
                if tg0 + TGB < (ig + 1) * TPG:
                    yield

        def drain(gen):
            if gen is not None:
                for _ in gen:
                    pass

        # ---------------- emission schedule ----------------
        # proj(bb0) first; then each scores block overlaps the next batch's
        # projections on the PE while ACT/DVE chew on exp/mult.
        with tc.tile_pool(name="qt", bufs=CC + 4) as qt_pool, \
             tc.tile_pool(name="pj", bufs=2, space="PSUM") as pj_pool:
            po_pool = pj_pool
            blocks = [(pair, ig, lb)
                      for pair in range(NPAIR)
                      for lb in range(2)
                      for ig in range(NIG)]
            # background projection stream: bb1, bb2, ... (bb0 emitted eagerly)
            drain(emit_proj_batch(0, qt_pool, pj_pool))
            state = {"bg": None, "bg_bb": -1, "next_bb": 1, "tail": None}

            def bg_start():
                if state["bg"] is None and state["next_bb"] < b:
                    state["bg_bb"] = state["next_bb"]
                    state["bg"] = emit_proj_batch(state["next_bb"], qt_pool, pj_pool)
                    state["next_bb"] += 1

            def ensure_projected(need_bb):
                # fully emit every projection batch <= need_bb before the
                # dependent score block enters the engine streams
                while state["bg_bb"] <= need_bb and (
                        state["bg"] is not None or state["next_bb"] <= need_bb):
                    bg_start()
                    drain(state["bg"])
                    state["bg"] = None
                    if state["next_bb"] <= need_bb:
                        continue
                    break

            def feed(jt, blk):
                # previous block's tail: evac/recip units right away (they
                # free the ot accumulators), but hold the output-projection
                # units until the reciprocal had time to land -- otherwise
                # its PSUM matmuls clog the PE wait queue while blocked on
                # recip, starving the new block's qk matmuls.
                if state["tail"] is not None and (jt < 2 or jt >= 5):
                    if next(state["tail"], StopIteration) is StopIteration:
                        state["tail"] = None
                # trickle the background projections: the PE only has
                # ~0.2us/jt of slack during a score block, so one unit per
                # jt in the first block (bb1 must finish in time) and one
                # every other jt after that
                if blk == 0 or jt % 2 == 0:
                    bg_start()
                    if state["bg"] is not None:
                        if next(state["bg"], StopIteration) is StopIteration:
                            state["bg"] = None

            for blk, (pair, ig, lb) in enumerate(blocks):
                ensure_projected(2 * pair + lb)
                box = {}
                for _ in emit_scores_main(pair, ig, lb, box):
                    feed(box["jt"], blk)
                drain(state["tail"])
                state["tail"] = emit_scores_tail(
                    pair, ig, lb, box, po_pool,
                    is_last=(pair, ig, lb) == blocks[-1])
            drain(state["tail"])
            drain(state["bg"])
            state["bg"] = None
            while state["next_bb"] < b:
                bg_start()
                drain(state["bg"])
                state["bg"] = None
    nc.compile()
    return nc


def make_in_maps(query, pos_bias, Wq, Wk, Wv, Wout, n_cores=N_CORES):
    """Host-side sharding/layout prep. Head h -> core h."""
    query = np.asarray(query, dtype=np.float32)
    pos_bias = np.asarray(pos_bias, dtype=np.float32)
    Wq = np.asarray(Wq, dtype=np.float32)
    Wk = np.asarray(Wk, dtype=np.float32)
    Wv = np.asarray(Wv, dtype=np.float32)
    Wout = np.asarray(Wout, dtype=np.float32)

    b, n, d = query.shape
    qT = np.ascontiguousarray(query.reshape(b * n, d).T).astype(BF)
    wq_s = (Wq * np.float32(SCALE)).astype(BF)
    wk_b = Wk.astype(BF)
    wv_b = Wv.astype(BF)
    in_maps = []
    for h in range(n_cores):
        sl = slice(h * DH, (h + 1) * DH)
        in_maps.append({
            "qT": qT,
            "ebiasT": np.ascontiguousarray(np.exp(pos_bias[h]).T.astype(BF)),
            "wq": np.ascontiguousarray(wq_s[:, sl]),
            "wk": np.ascontiguousarray(wk_b[:, sl]),
            "wv": np.ascontiguousarray(wv_b[:, sl]),
            "wout": np.ascontiguousarray(Wout[sl, :]),
        })
    return in_maps


def run_device(in_maps, b=B, n=N, d=D, packed=False, trace=False, **kw):
    nc = build_nc(b, n, d, packed, n_cores=len(in_maps))
    return run_bass_kernel_spmd(nc, in_maps, list(range(len(in_maps))), trace=trace, **kw)


def assemble(results, b=B, n=N, d=D):
    acc = np.zeros((b * n, d), dtype=np.float32)
    for r in results:
        acc += r["out"].astype(np.float32)
    return acc.reshape(b, n, d)


def kernel(query, pos_bias, Wq, Wk, Wv, Wout):
    in_maps = make_in_maps(query, pos_bias, Wq, Wk, Wv, Wout)
    res = run_device(in_maps)
    return assemble(res.results)
